# revision 1
# baseline (speedup 1.0000x reference)
"""Trainium2 Bass kernel for nn_GCIQEValue (MLP + IQE head), 8-core data parallel.

Math (validated vs reference):
  phi(x) = LN-MLP: 3x [matmul+bias -> tanh-gelu -> LayerNorm(affine folded into
  next W on host)] then final matmul+bias.
  IQE per row, per 32-dim component c with x = phi_s[c], y = phi_g[c]:
    y' = max(x, y)                      (interval [x_i, max(x_i,y_i)])
    u = sort(x), v = sort(y')           (independent keys-only sorts: the
                                         union-measure depends only on the
                                         multisets of starts/ends)
    comp_c = sum(v) - u_0 - sum_{i>=1} max(u_i, v_{i-1})
  out = sig(alpha) * mean_c(comp) + (1 - sig(alpha)) * max_c(comp)

Structure: 7-stage software pipeline (For_i_pipelined) over 128-row tiles:
  S0 load | S1 L0 | S2 L1 | S3 L2 | S4 L3+ymax | S5 sort p0-7 | S6 sort p8-14+post
LN statistics ride ACT accum_out (sum of gelu / sum of squares); the bitonic
sort runs on DVE as strided min/max tensor_tensor pairs.
"""

import numpy as np

B = 131072
OBS = 64
H = 512
NCOMP = 16
DPC = 32
NCORES = 8
P = 128
LN_EPS = 1e-6

_CACHE = {}

# bitonic schedule for 32-wide ascending sort: 15 passes
_SCHED = [("pair", 0, 0)]
for _L in (4, 8, 16, 32):
    _SCHED.append(("flip", _L, 0))
    _d = _L // 4
    while _d >= 1:
        _SCHED.append(("shift", _L, _d))
        _d //= 2


# ---------------------------------------------------------------- device kernel
def build_nc(rows_per_core=B // NCORES, unroll=4, gelu="hw", repeats=1,
             stage_bufs=None, mlp_bufs=3, psum_bufs=4, split_pass=7,
             n_passes=15, sort_chunks=1, n_layers=3, ln_lite=False,
             rsqrt_newton=True, bias_mode="pe", split_l0=True, hints=False):
    """Build the Bass (Bacc) module for one core processing rows_per_core rows."""
    import concourse.bass as bass
    import concourse.mybir as mybir
    import concourse.tile as tile
    from concourse import bacc
    from concourse.masks import make_identity

    fp32 = mybir.dt.float32
    AT = mybir.ActivationFunctionType
    OP = mybir.AluOpType

    nt = rows_per_core // P
    assert rows_per_core % P == 0
    if stage_bufs is None:
        stage_bufs = unroll

    nc = bacc.Bacc("TRN2", target_bir_lowering=False, debug=False)

    obs = nc.declare_dram_parameter("observations", [rows_per_core, OBS], fp32,
                                    isOutput=False)
    gls = nc.declare_dram_parameter("goals", [rows_per_core, OBS], fp32,
                                    isOutput=False)
    w0d = nc.declare_dram_parameter("w0", [OBS, H], fp32, isOutput=False)
    w1d = nc.declare_dram_parameter("w1", [H, H], fp32, isOutput=False)
    w2d = nc.declare_dram_parameter("w2", [H, H], fp32, isOutput=False)
    w3d = nc.declare_dram_parameter("w3", [H, H], fp32, isOutput=False)
    bsd = nc.declare_dram_parameter("bs", [4, H], fp32, isOutput=False)
    avd = nc.declare_dram_parameter("avec", [P, 2], fp32, isOutput=False)
    out = nc.declare_dram_parameter("out", [rows_per_core], fp32, isOutput=True)

    obs_v = obs[:].rearrange("(n p) f -> n p f", p=P)
    gls_v = gls[:].rearrange("(n p) f -> n p f", p=P)
    out_v = out[:].rearrange("(n p) -> n p", p=P)

    gelu_f = AT.Gelu_apprx_tanh if gelu == "hw" else AT.Identity

    with tile.TileContext(nc) as tc:
        with (
            tc.tile_pool(name="const", bufs=1) as cpool,
            tc.tile_pool(name="mlp", bufs=mlp_bufs) as mp,
            tc.tile_pool(name="srt", bufs=mlp_bufs) as sp,
            tc.tile_pool(name="pipe", bufs=1) as pipe_pool,
            tc.tile_pool(name="ps", bufs=psum_bufs, space="PSUM") as pp,
            tc.tile_pool(name="pst", bufs=8 - psum_bufs, space="PSUM") as ppt,
        ):
            # ---- constants
            w0 = cpool.tile([OBS, H], fp32)
            nc.sync.dma_start(out=w0, in_=w0d[:])
            wl = []
            for wd, nm in ((w1d, "w1"), (w2d, "w2"), (w3d, "w3")):
                t = cpool.tile([P, 4, H], fp32, tag=nm)
                nc.sync.dma_start(out=t, in_=wd[:].rearrange("(c p) n -> p c n", p=P))
                wl.append(t)
            bsc = cpool.tile([1, 4, H], fp32)
            nc.sync.dma_start(out=bsc, in_=bsd[:].rearrange("(o c) n -> o c n", o=1))
            if bias_mode != "pe":
                bsb = cpool.tile([P, 4, H], fp32)
                nc.sync.dma_start(
                    out=bsb,
                    in_=bass.AP(tensor=bsd[:].tensor, offset=0,
                                ap=[[0, P]] + list(bsd[:].ap)))
            avec = cpool.tile([P, 2], fp32)
            nc.sync.dma_start(out=avec, in_=avd[:])
            ident = cpool.tile([P, P], fp32)
            make_identity(nc, ident)
            ones = cpool.tile([1, P], fp32)
            nc.vector.memset(ones, 1.0)
            epst = cpool.tile([P, 1], fp32)
            nc.vector.memset(epst, LN_EPS)

            def matmul_from(t_sb, li):
                """t_sb [128, F_in] row-major -> pz PSUM [128, 512] for layer li
                (li = 0 uses w0/64-wide input, else wl[li-1])."""
                pz = pp.tile([P, H], fp32, tag="pz")
                start = True
                if bias_mode != "pe":
                    eng = nc.scalar if bias_mode == "act" else nc.vector
                    if bias_mode == "act":
                        eng.copy(pz, bsb[:, li, :])
                    else:
                        eng.tensor_copy(pz, bsb[:, li, :])
                    start = False
                if li == 0:
                    pTf = ppt.tile([P, H], fp32, tag="pT")
                    nc.tensor.transpose(pTf[0:OBS, 0:P], t_sb, ident)
                    xT = mp.tile([OBS, P], fp32, tag="xT")
                    nc.scalar.copy(xT, pTf[0:OBS, 0:P])
                    nc.tensor.matmul(pz, xT, w0, start=start,
                                     stop=(bias_mode != "pe"))
                else:
                    pTf = ppt.tile([P, H], fp32, tag="pT")
                    for k in range(4):
                        nc.tensor.transpose(pTf[:, k * P:(k + 1) * P],
                                            t_sb[:, k * P:(k + 1) * P], ident)
                    tT = mp.tile([P, 4, P], fp32, tag="tT")
                    nc.scalar.copy(tT, pTf)
                    for k in range(4):
                        nc.tensor.matmul(pz, tT[:, k, :], wl[li - 1][:, k, :],
                                         start=(start and k == 0),
                                         stop=(bias_mode != "pe" and k == 3))
                if bias_mode == "pe":
                    nc.tensor.matmul(pz, ones, bsc[:, li, :], start=False,
                                     stop=True)
                return pz

            def gelu_ln(pz, t_out):
                """pz PSUM -> t_out SBUF: LayerNorm(gelu(pz)) via ACT stats."""
                if ln_lite:
                    nc.scalar.activation(t_out, pz, gelu_f)
                    return
                g = mp.tile([P, H], fp32, tag="g")
                sums = mp.tile([P, 2], fp32, tag="sums")
                nc.scalar.activation(g, pz, gelu_f, accum_out=sums[:, 0:1])
                gsq = mp.tile([P, H], fp32, tag="gsq")
                nc.scalar.activation(gsq, g, AT.Square, accum_out=sums[:, 1:2])
                mv2 = mp.tile([P, 2], fp32, tag="mv2")
                nc.vector.tensor_scalar_mul(mv2, sums, 1.0 / H)
                msq = mp.tile([P, 1], fp32, tag="msq")
                nc.vector.tensor_tensor(out=msq, in0=mv2[:, 0:1],
                                        in1=mv2[:, 0:1], op=OP.mult)
                varb = mp.tile([P, 1], fp32, tag="varb")
                nc.vector.tensor_tensor(out=varb, in0=mv2[:, 1:2], in1=msq,
                                        op=OP.subtract)
                nc.vector.tensor_scalar_add(varb, varb, LN_EPS)
                rstd = mp.tile([P, 1], fp32, tag="rstd")
                if rsqrt_newton:
                    # rsqrt without the ACT Sqrt table set: quake seed on DVE
                    # int ALU + 3 Newton iterations (rel err ~1e-7).
                    i32 = mybir.dt.int32
                    yi = mp.tile([P, 1], i32, tag="yi")
                    nc.vector.tensor_scalar(
                        out=yi, in0=varb.bitcast(i32), scalar1=1,
                        scalar2=None, op0=OP.logical_shift_right)
                    nc.vector.tensor_scalar(
                        out=yi, in0=yi, scalar1=-1, scalar2=0x5F3759DF,
                        op0=OP.mult, op1=OP.add)
                    y = yi.bitcast(fp32)
                    t1 = mp.tile([P, 1], fp32, tag="nt1")
                    for _ in range(3):
                        nc.vector.tensor_tensor(out=t1, in0=varb, in1=y,
                                                op=OP.mult)
                        nc.vector.tensor_tensor(out=t1, in0=t1, in1=y,
                                                op=OP.mult)
                        nc.vector.tensor_scalar(out=t1, in0=t1, scalar1=-0.5,
                                                scalar2=1.5, op0=OP.mult,
                                                op1=OP.add)
                        nc.vector.tensor_tensor(out=y, in0=y, in1=t1,
                                                op=OP.mult)
                    rstd = y
                else:
                    std = mp.tile([P, 1], fp32, tag="std")
                    nc.scalar.activation(std, varb, AT.Sqrt)
                    nc.vector.reciprocal(rstd, std)
                nmr = mp.tile([P, 1], fp32, tag="nmr")
                nc.vector.scalar_tensor_tensor(out=nmr, in0=mv2[:, 0:1],
                                               scalar=-1.0, in1=rstd,
                                               op0=OP.mult, op1=OP.mult)
                nc.scalar.activation(t_out, g, AT.Identity, bias=nmr, scale=rstd)

            def emit_sort_pass(p_idx, src_x, src_y, dst):
                """Emit bitonic pass p_idx. Pass 0 reads (src_x, src_y) pair
                tensors; later passes read src_x as the full [P,1024] buffer.
                sort_chunks splits each instruction along the group dim to
                amortize the DVE post-op DRAIN (cost ~ 2*dur - const)."""
                kind, L, d = _SCHED[p_idx]
                V = nc.vector
                C = sort_chunks

                def ch(view, c):
                    n = view.shape[1]
                    return view[:, c * n // C:(c + 1) * n // C]

                if kind == "pair":
                    for src, off in ((src_x, 0), (src_y, H)):
                        s = src.rearrange("p (g e) -> p g e", e=DPC)
                        o = dst[:, off:off + H].rearrange("p (g e) -> p g e",
                                                          e=DPC)
                        for c in range(C):
                            sc, oc = ch(s, c), ch(o, c)
                            V.tensor_tensor(out=oc[:, :, 0::2],
                                            in0=sc[:, :, 0::2],
                                            in1=sc[:, :, 1::2], op=OP.min)
                            V.tensor_tensor(out=oc[:, :, 1::2],
                                            in0=sc[:, :, 0::2],
                                            in1=sc[:, :, 1::2], op=OP.max)
                elif kind == "flip":
                    half = L // 2
                    s = src_x.rearrange("p (b e) -> p b e", e=L)
                    o = dst.rearrange("p (b e) -> p b e", e=L)
                    for c in range(C):
                        sc, oc = ch(s, c), ch(o, c)
                        V.tensor_tensor(out=oc[:, :, 0:half],
                                        in0=sc[:, :, 0:half],
                                        in1=sc[:, :, L - 1:half - 1:-1],
                                        op=OP.min)
                        V.tensor_tensor(out=oc[:, :, half:L],
                                        in0=sc[:, :, half:L],
                                        in1=sc[:, :, half - 1::-1], op=OP.max)
                else:
                    s = src_x.rearrange("p (c e) -> p c e", e=2 * d)
                    o = dst.rearrange("p (c e) -> p c e", e=2 * d)
                    for c in range(C):
                        sc, oc = ch(s, c), ch(o, c)
                        V.tensor_tensor(out=oc[:, :, 0:d], in0=sc[:, :, 0:d],
                                        in1=sc[:, :, d:2 * d], op=OP.min)
                        V.tensor_tensor(out=oc[:, :, d:2 * d],
                                        in0=sc[:, :, 0:d],
                                        in1=sc[:, :, d:2 * d], op=OP.max)

            # ---------------- pipeline stages
            def st_load(pipe, iv):
                xt = pipe.intermediate_tile([P, OBS], fp32, name="xt")
                gt = pipe.intermediate_tile([P, OBS], fp32, name="gt")
                nc.sync.dma_start(out=xt, in_=obs_v[iv])
                nc.sync.dma_start(out=gt, in_=gls_v[iv])
                return (xt, gt)

            def mk_layer(li):
                def st(pipe, iv, prev):
                    to, tg = prev
                    if n_layers < 3:  # ablation: copy-through this mid layer
                        oo = pipe.intermediate_tile([P, H], fp32, name=f"to{li}")
                        og = pipe.intermediate_tile([P, H], fp32, name=f"tg{li}")
                        nc.scalar.copy(oo, to)
                        nc.scalar.copy(og, tg)
                        return (oo, og)
                    oo = pipe.intermediate_tile([P, H], fp32, name=f"to{li}")
                    og = pipe.intermediate_tile([P, H], fp32, name=f"tg{li}")
                    gelu_ln(matmul_from(to, li), oo)
                    gelu_ln(matmul_from(tg, li), og)
                    return (oo, og)
                return st

            def st_l3(pipe, iv, prev):
                to, tg = prev
                phis = pipe.intermediate_tile([P, H], fp32, name="phis")
                pz = matmul_from(to, 3)
                nc.scalar.copy(phis, pz)
                pzg = matmul_from(tg, 3)
                ypr = pipe.intermediate_tile([P, H], fp32, name="ypr")
                nc.vector.tensor_tensor(out=ypr, in0=phis, in1=pzg, op=OP.max)
                return (phis, ypr)

            def st_sort_a(pipe, iv, prev):
                phis, ypr = prev
                bufA = pipe.intermediate_tile([P, 2 * H], fp32, name="bufA")
                bufB = pipe.intermediate_tile([P, 2 * H], fp32, name="bufB")
                emit_sort_pass(0, phis, ypr, bufA)
                cur, nxt = bufA, bufB
                for pidx in range(1, split_pass):
                    if pidx < n_passes:
                        emit_sort_pass(pidx, cur, None, nxt)
                    cur, nxt = nxt, cur
                return (bufA, bufB)

            def st_sort_b(pipe, iv, prev):
                bufA, bufB = prev
                cur, nxt = (bufB, bufA) if split_pass % 2 == 0 else (bufA, bufB)
                for pidx in range(split_pass, 15):
                    if pidx < n_passes:
                        emit_sort_pass(pidx, cur, None, nxt)
                    cur, nxt = nxt, cur
                fin = cur  # pass 14 (even) -> bufA when split parity works out
                fv = fin.rearrange("p (h g e) -> p h g e", h=2, e=DPC)
                # coupling: u[i] <- max(u[i], v[i-1]) for i>=1, in place
                nc.vector.tensor_tensor(out=fv[:, 0, :, 1:DPC],
                                        in0=fv[:, 0, :, 1:DPC],
                                        in1=fv[:, 1, :, 0:DPC - 1], op=OP.max)
                red = sp.tile([P, 2, NCOMP], fp32, tag="red")
                nc.vector.tensor_reduce(out=red, in_=fv,
                                        axis=mybir.AxisListType.X, op=OP.add)
                comp = sp.tile([P, NCOMP], fp32, tag="comp")
                nc.vector.tensor_tensor(out=comp, in0=red[:, 1, :],
                                        in1=red[:, 0, :], op=OP.subtract)
                cs = sp.tile([P, 1], fp32, tag="cs")
                nc.vector.tensor_reduce(out=cs, in_=comp,
                                        axis=mybir.AxisListType.X, op=OP.add)
                cm = sp.tile([P, 1], fp32, tag="cm")
                nc.vector.tensor_reduce(out=cm, in_=comp,
                                        axis=mybir.AxisListType.X, op=OP.max)
                res = sp.tile([P, 1], fp32, tag="res")
                nc.vector.tensor_scalar(out=res, in0=cs, scalar1=avec[:, 0:1],
                                        scalar2=None, op0=OP.mult)
                nc.vector.scalar_tensor_tensor(out=res, in0=cm,
                                               scalar=avec[:, 1:2], in1=res,
                                               op0=OP.mult, op1=OP.add)
                nc.sync.dma_start(out=out_v[iv], in_=res[:, 0:1])

            def st_l01(pipe, iv, prev):
                xt, gt = prev
                t0o = mp.tile([P, H], fp32, tag="t0o")
                t0g = mp.tile([P, H], fp32, tag="t0g")
                gelu_ln(matmul_from(xt, 0), t0o)
                gelu_ln(matmul_from(gt, 0), t0g)
                oo = pipe.intermediate_tile([P, H], fp32, name="to1")
                og = pipe.intermediate_tile([P, H], fp32, name="tg1")
                gelu_ln(matmul_from(t0o, 1), oo)
                gelu_ln(matmul_from(t0g, 1), og)
                return (oo, og)

            def st_l0(pipe, iv, prev):
                xt, gt = prev
                oo = pipe.intermediate_tile([P, H], fp32, name="to0")
                og = pipe.intermediate_tile([P, H], fp32, name="tg0")
                gelu_ln(matmul_from(xt, 0), oo)
                gelu_ln(matmul_from(gt, 0), og)
                return (oo, og)

            if split_l0:
                stages = [st_load, st_l0, mk_layer(1), mk_layer(2), st_l3,
                          st_sort_a, st_sort_b]
            else:
                stages = [st_load, st_l01, mk_layer(2), st_l3,
                          st_sort_a, st_sort_b]

            def run_pipe():
                he = (mybir.EngineType.PE, mybir.EngineType.DVE,
                      mybir.EngineType.Activation, mybir.EngineType.SP,
                      mybir.EngineType.Pool) if hints else ()
                tc.For_i_pipelined(stages, 0, nt, 1, pool=pipe_pool,
                                   unroll=unroll, staged_num_bufs=stage_bufs,
                                   hint_engines=he)

            if repeats == 1:
                run_pipe()
            else:
                with tc.For_i(0, repeats, 1):
                    run_pipe()

    nc.finalize()
    return nc


# ---------------------------------------------------------------- host wrapper
def _prep_host(inputs):
    """Fold LN affine params into the following layer's weights; build avec."""
    f32 = np.float32
    W0 = np.asarray(inputs["W0"], f32)
    b0 = np.asarray(inputs["b0"], f32)
    w, b = [W0], [b0]
    for i in (0, 1, 2):
        s = np.asarray(inputs[f"ln{i}_s"], f32)
        t = np.asarray(inputs[f"ln{i}_b"], f32)
        Wn = np.asarray(inputs[("W1", "W2", "W3")[i]], f32)
        bn = np.asarray(inputs[("b1", "b2", "b3")[i]], f32)
        w.append(s[:, None] * Wn)
        b.append(bn + t @ Wn)
    bs = np.stack(b, 0)  # [4, 512]
    alpha = float(np.asarray(inputs["alpha"]))
    a = 1.0 / (1.0 + np.exp(-alpha))
    avec = np.empty((P, 2), f32)
    avec[:, 0] = a / NCOMP
    avec[:, 1] = 1.0 - a
    return w[0], w[1], w[2], w[3], bs.astype(f32), avec


def _probe_devices():
    """Poke every core with a tiny op; retries to shake off a stale
    NRT_EXEC_UNIT_UNRECOVERABLE state left by a previous process."""
    import jax
    import jax.numpy as jnp

    for attempt in range(3):
        try:
            for d in jax.devices()[:NCORES]:
                jnp.zeros((1,), jnp.float32, device=d).block_until_ready()
            return
        except Exception:
            if attempt == 2:
                raise


def run_on_device(inputs, rows_total=B, trace=False, repeats=1, **build_kw):
    """Shard, run on 8 cores, gather. Returns (out [rows_total], results obj)."""
    from concourse.bass_utils import run_bass_kernel_spmd

    _probe_devices()

    rows_core = rows_total // NCORES
    key = (rows_core, repeats, tuple(sorted(build_kw.items())))
    if key not in _CACHE:
        _CACHE[key] = build_nc(rows_core, repeats=repeats, **build_kw)
    nc = _CACHE[key]

    w0, w1, w2, w3, bs, avec = _prep_host(inputs)
    ob = np.ascontiguousarray(np.asarray(inputs["observations"], np.float32)[:rows_total])
    gl = np.ascontiguousarray(np.asarray(inputs["goals"], np.float32)[:rows_total])
    in_maps = []
    for c in range(NCORES):
        sl = slice(c * rows_core, (c + 1) * rows_core)
        in_maps.append({
            "observations": ob[sl], "goals": gl[sl],
            "w0": w0, "w1": w1, "w2": w2, "w3": w3, "bs": bs, "avec": avec,
        })
    r = run_bass_kernel_spmd(nc, in_maps, list(range(NCORES)), trace=trace)
    outp = np.concatenate([r.results[c]["out"] for c in range(NCORES)])
    return outp, r


def kernel(**inputs):
    out, _ = run_on_device(inputs)
    return out.astype(np.float32)



# revision 24
# speedup vs baseline: 3.1064x; 3.1064x over previous
"""Trainium2 Bass kernel for nn_GCIQEValue (MLP + IQE head), 8-core data parallel.

Math (validated vs reference):
  phi(x) = LN-MLP: 3x [matmul+bias -> tanh-gelu -> LayerNorm(affine folded into
  next W on host)] then final matmul+bias.
  IQE per row, per 32-dim component c with x = phi_s[c], y = phi_g[c]:
    y' = max(x, y)                      (interval [x_i, max(x_i,y_i)])
    u = sort(x), v = sort(y')           (independent keys-only sorts)
    comp_c = sum(v) - u_0 - sum_{i>=1} max(u_i, v_{i-1})
  out = sig(alpha) * mean_c(comp) + (1 - sig(alpha)) * max_c(comp)

v2: fp16 weights/activations/transposes (PE fp32 matmul runs at 1/4 rate) and
fp16 bitonic sort (DVE 2x mode on aligned shift passes). When the effective
biases are zero (structurally true for this generator), the LayerNorm affine
is folded forward: LN(g) @ W == rstd * (g@W - m*colsum(W)), so the per-row
scale rides the next layer's gelu `scale` operand, the -m*colsum(W) term is
one K=1 matmul row, and the standalone scale-apply ACT op disappears.
Squares for the LN variance run on the otherwise-idle GPSIMD engine. LN
stats are batched across the obs/goals streams (one [P,2] Newton-rsqrt chain
per layer). The compare+reduce tail is fused via tensor_tensor_reduce.

Structure: 7-stage software pipeline (For_i_pipelined) over 128-row tiles:
  S0 load | S1 L0 | S2 L1 | S3 L2 | S4 L3+ymax | S5 sort p0-6 | S6 sort p7-14+post
"""

import numpy as np

B = 131072
OBS = 64
H = 512
NCOMP = 16
DPC = 32
NCORES = 8
P = 128
LN_EPS = 1e-6

_CACHE = {}

# bitonic schedule for 32-wide ascending sort: 15 passes
_SCHED = [("pair", 0, 0)]
for _L in (4, 8, 16, 32):
    _SCHED.append(("flip", _L, 0))
    _d = _L // 4
    while _d >= 1:
        _SCHED.append(("shift", _L, _d))
        _d //= 2


# ---------------------------------------------------------------- device kernel
def build_nc(rows_per_core=B // NCORES, unroll=4, repeats=1,
             stage_bufs=None, mlp_bufs=3, psum_bufs=4, split_pass=7,
             n_passes=15, newton=2, sort16=1, mm16=1, has_bias=0,
             gsq_pool=0, fuse_tail=1, n_layers=3, ln_lite=False, hints=False):
    """Build the Bass (Bacc) module for one core processing rows_per_core rows.

    has_bias=0 uses the folded-LN fast path (valid only when effective biases
    are zero); has_bias=1 keeps a standalone scale-apply ACT op per stream.
    """
    import concourse.bass as bass
    import concourse.mybir as mybir
    import concourse.tile as tile
    from concourse import bacc
    from concourse.masks import make_identity

    fp32 = mybir.dt.float32
    i32 = mybir.dt.int32
    adt = mybir.dt.float16 if mm16 else fp32   # activations/weights
    sdt = mybir.dt.float16 if sort16 else fp32  # sort buffers
    AT = mybir.ActivationFunctionType
    OP = mybir.AluOpType
    fold = (not has_bias) and mm16  # folded path needs the fp16 bank layout

    nt = rows_per_core // P
    assert rows_per_core % P == 0
    if stage_bufs is None:
        stage_bufs = unroll

    nc = bacc.Bacc("TRN2", target_bir_lowering=False, debug=False)

    obs = nc.declare_dram_parameter("observations", [rows_per_core, OBS], adt,
                                    isOutput=False)
    gls = nc.declare_dram_parameter("goals", [rows_per_core, OBS], adt,
                                    isOutput=False)
    w0d = nc.declare_dram_parameter("w0", [OBS, H], adt, isOutput=False)
    w1d = nc.declare_dram_parameter("w1", [H, H], adt, isOutput=False)
    w2d = nc.declare_dram_parameter("w2", [H, H], adt, isOutput=False)
    w3d = nc.declare_dram_parameter("w3", [H, H], adt, isOutput=False)
    cwd = nc.declare_dram_parameter("csw", [3, H], adt, isOutput=False)
    if has_bias:
        bsd = nc.declare_dram_parameter("bs", [4, H], adt, isOutput=False)
    avd = nc.declare_dram_parameter("avec", [P, 2], fp32, isOutput=False)
    out = nc.declare_dram_parameter("out", [rows_per_core], fp32, isOutput=True)

    obs_v = obs[:].rearrange("(n p) f -> n p f", p=P)
    gls_v = gls[:].rearrange("(n p) f -> n p f", p=P)
    out_v = out[:].rearrange("(n p) -> n p", p=P)

    with tile.TileContext(nc) as tc:
        with (
            tc.tile_pool(name="const", bufs=1) as cpool,
            tc.tile_pool(name="mlp", bufs=mlp_bufs) as mp,
            tc.tile_pool(name="srt", bufs=mlp_bufs) as sp,
            tc.tile_pool(name="pipe", bufs=1) as pipe_pool,
            tc.tile_pool(name="ps", bufs=psum_bufs, space="PSUM") as pp,
            tc.tile_pool(name="pst", bufs=8 - psum_bufs, space="PSUM") as ppt,
        ):
            # ---- constants
            w0 = cpool.tile([OBS, H], adt)
            nc.sync.dma_start(out=w0, in_=w0d[:])
            wl = []
            for wd, nm in ((w1d, "w1"), (w2d, "w2"), (w3d, "w3")):
                t = cpool.tile([P, 4, H], adt, tag=nm)
                nc.sync.dma_start(out=t, in_=wd[:].rearrange("(c p) n -> p c n", p=P))
                wl.append(t)
            # csw rows duplicated at partitions 0 and 32 (one per stream) so
            # the K=1 -m-row matmuls satisfy lhsT/rhs base-partition matching
            csw = cpool.tile([33, 3, H], adt)
            nc.sync.dma_start(out=csw[0:1, :, :],
                              in_=cwd[:].rearrange("(o c) n -> o c n", o=1))
            nc.sync.dma_start(out=csw[32:33, :, :],
                              in_=cwd[:].rearrange("(o c) n -> o c n", o=1))
            if has_bias:
                bsc = cpool.tile([1, 4, H], adt)
                nc.sync.dma_start(out=bsc, in_=bsd[:].rearrange("(o c) n -> o c n", o=1))
                ones = cpool.tile([1, P], adt)
                nc.vector.memset(ones, 1.0)
            avec = cpool.tile([P, 2], fp32)
            nc.sync.dma_start(out=avec, in_=avd[:])
            ident = cpool.tile([P, P], adt)
            make_identity(nc, ident)

            # PT tile holds the 4 transposed 128-chunks of one stream plus,
            # in its tail columns, the transposed -m rows of this layer.
            # Sized to exactly one 2KB PSUM bank (fp16) so ring slots never
            # share a zero region with the pending pz accumulation group.
            PTW = 2 * H if mm16 else H  # one full 2KB PSUM bank either way

            def matmuls(t_sb, li, negm_row, pTf):
                """t_sb [128, F_in] adt -> pz PSUM fp32 [128, 512] for layer li.
                negm_row: [1, P] lhsT adding -m * colsum(W') when folding."""
                pz = pp.tile([P, H], fp32, tag="pz")
                if li == 0:
                    nc.tensor.transpose(pTf[0:OBS, 0:P], t_sb, ident)
                    xT = mp.tile([OBS, P], adt, tag="xT")
                    nc.scalar.copy(xT, pTf[0:OBS, 0:P])
                    nc.tensor.matmul(pz, xT, w0, start=True,
                                     stop=(not has_bias))
                else:
                    for k in range(4):
                        nc.tensor.transpose(pTf[:, k * P:(k + 1) * P],
                                            t_sb[:, k * P:(k + 1) * P], ident)
                    tT = mp.tile([P, 4, P], adt, tag="tT")
                    nc.scalar.copy(tT, pTf[:, 0:H])
                    last = 3 if negm_row is None else 4
                    for k in range(4):
                        nc.tensor.matmul(pz, tT[:, k, :], wl[li - 1][:, k, :],
                                         start=(k == 0),
                                         stop=(not has_bias and k == last))
                    if negm_row is not None:
                        bp = negm_row.base_partition()
                        nc.tensor.matmul(pz, negm_row,
                                         csw[bp:bp + 1, li - 1, :],
                                         start=False, stop=(not has_bias))
                if has_bias:
                    nc.tensor.matmul(pz, ones, bsc[:, li, :], start=False,
                                     stop=True)
                return pz

            def ln_stats(g, sums, pz_o, pz_g, pTf):
                """Row stats of the raw-gelu pair g [P,2,H] (gelu sums in
                `sums`): returns (rstd [P,2] fp32, pTf holding [2,P] -m^T) for
                the next layer. Non-fold path instead rescales g in place."""
                sq = mp.tile([P, 2], fp32, tag="sq")
                if gsq_pool:
                    # squares on the idle GPSIMD; per-row sum on DVE (GpSimd
                    # tensor_reduce is partition-axis only)
                    gsq = mp.tile([P, 2, H], adt, tag="gsq")
                    nc.gpsimd.tensor_tensor(out=gsq, in0=g, in1=g, op=OP.mult)
                    sq2 = mp.tile([P, 2], fp32, tag="sq2")
                    nc.vector.tensor_reduce(out=sq2, in_=gsq,
                                            axis=mybir.AxisListType.X,
                                            op=OP.add)
                    nc.vector.tensor_scalar_mul(sq, sq2, 1.0 / H)
                else:
                    nc.scalar.activation(pz_o, g[:, 0, :], AT.Square,
                                         accum_out=sq[:, 0:1])
                    nc.scalar.activation(pz_g, g[:, 1, :], AT.Square,
                                         accum_out=sq[:, 1:2])
                    nc.vector.tensor_scalar_mul(sq, sq, 1.0 / H)
                m = mp.tile([P, 2], fp32, tag="m")
                nc.vector.tensor_scalar_mul(m, sums, 1.0 / H)
                varb = mp.tile([P, 2], fp32, tag="varb")
                nc.vector.tensor_tensor(out=varb, in0=m, in1=m, op=OP.mult)
                nc.vector.tensor_tensor(out=varb, in0=sq, in1=varb,
                                        op=OP.subtract)
                nc.vector.tensor_scalar_add(varb, varb, LN_EPS)
                # rsqrt: quake seed on int ALU + `newton` NR iterations
                yi = mp.tile([P, 2], i32, tag="yi")
                nc.vector.tensor_scalar(
                    out=yi, in0=varb.bitcast(i32), scalar1=1,
                    scalar2=None, op0=OP.logical_shift_right)
                nc.vector.tensor_scalar(
                    out=yi, in0=yi, scalar1=-1, scalar2=0x5F3759DF,
                    op0=OP.mult, op1=OP.add)
                y = yi.bitcast(fp32)
                t1 = mp.tile([P, 2], fp32, tag="nt1")
                for _ in range(newton):
                    nc.vector.tensor_tensor(out=t1, in0=varb, in1=y, op=OP.mult)
                    nc.vector.tensor_tensor(out=t1, in0=t1, in1=y, op=OP.mult)
                    nc.vector.tensor_scalar(out=t1, in0=t1, scalar1=-0.5,
                                            scalar2=1.5, op0=OP.mult,
                                            op1=OP.add)
                    nc.vector.tensor_tensor(out=y, in0=y, in1=t1, op=OP.mult)
                if fold:
                    mh = mp.tile([P, 2], adt, tag="mh")
                    nc.vector.tensor_scalar(out=mh, in0=m, scalar1=-1.0,
                                            scalar2=None, op0=OP.mult)
                    # matmul lhsT rows must sit at base partition 0/32/64
                    nc.tensor.transpose(pTf[0:1, H:H + P], mh[:, 0:1], ident)
                    nc.tensor.transpose(pTf[32:33, H:H + P], mh[:, 1:2], ident)
                    return y, pTf
                nmr = mp.tile([P, 2], fp32, tag="nmr")
                nc.vector.scalar_tensor_tensor(out=nmr, in0=m, scalar=-1.0,
                                               in1=y, op0=OP.mult, op1=OP.mult)
                for s in range(2):
                    nc.scalar.activation(g[:, s, :], g[:, s, :], AT.Identity,
                                         bias=nmr[:, s:s + 1],
                                         scale=y[:, s:s + 1])
                return y, None

            def emit_sort_pass(p_idx, src_x, src_y, dst):
                """Emit bitonic pass p_idx. Pass 0 reads (src_x, src_y) pair
                tensors; later passes read src_x as the full [P,1024] buffer."""
                kind, L, d = _SCHED[p_idx]
                V = nc.vector
                if kind == "pair":
                    for src, off in ((src_x, 0), (src_y, H)):
                        s = src.rearrange("p (g e) -> p g e", e=DPC)
                        o = dst[:, off:off + H].rearrange("p (g e) -> p g e",
                                                          e=DPC)
                        V.tensor_tensor(out=o[:, :, 0::2], in0=s[:, :, 0::2],
                                        in1=s[:, :, 1::2], op=OP.min)
                        V.tensor_tensor(out=o[:, :, 1::2], in0=s[:, :, 0::2],
                                        in1=s[:, :, 1::2], op=OP.max)
                elif kind == "flip":
                    half = L // 2
                    s = src_x.rearrange("p (b e) -> p b e", e=L)
                    o = dst.rearrange("p (b e) -> p b e", e=L)
                    V.tensor_tensor(out=o[:, :, 0:half], in0=s[:, :, 0:half],
                                    in1=s[:, :, L - 1:half - 1:-1], op=OP.min)
                    V.tensor_tensor(out=o[:, :, half:L], in0=s[:, :, half:L],
                                    in1=s[:, :, half - 1::-1], op=OP.max)
                else:
                    s = src_x.rearrange("p (c e) -> p c e", e=2 * d)
                    o = dst.rearrange("p (c e) -> p c e", e=2 * d)
                    V.tensor_tensor(out=o[:, :, 0:d], in0=s[:, :, 0:d],
                                    in1=s[:, :, d:2 * d], op=OP.min)
                    V.tensor_tensor(out=o[:, :, d:2 * d], in0=s[:, :, 0:d],
                                    in1=s[:, :, d:2 * d], op=OP.max)

            # ---------------- pipeline stages
            # inter-stage tuple for MLP stages: (g [P,2,H] raw gelu pair,
            # rstd [P,2] fp32, negmT_holder or None)
            def st_load(pipe, iv):
                xt = pipe.intermediate_tile([P, OBS], adt, name="xt")
                gt = pipe.intermediate_tile([P, OBS], adt, name="gt")
                nc.sync.dma_start(out=xt, in_=obs_v[iv])
                nc.sync.dma_start(out=gt, in_=gls_v[iv])
                return (xt, gt)

            def gelu_pair(pz_o, pz_g, g, rstd, sums):
                for s, pz in ((0, pz_o), (1, pz_g)):
                    kw = {}
                    if fold and rstd is not None:
                        kw["scale"] = rstd[:, s:s + 1]
                    nc.scalar.activation(g[:, s, :], pz, AT.Gelu_apprx_tanh,
                                         accum_out=sums[:, s:s + 1], **kw)

            def pipe_keep(pipe, y, name):
                t = pipe.intermediate_tile([P, 2], fp32, name=name)
                nc.vector.tensor_copy(t, y)
                return t

            def mk_layer(li):
                def st(pipe, iv, prev):
                    if li == 0:
                        srcs = prev
                        rstd_in = negmT_in = None
                    else:
                        g_in = prev[0]
                        rstd_in = prev[1] if len(prev) > 1 else None
                        negmT_in = prev[2] if len(prev) > 2 else None
                        srcs = (g_in[:, 0, :], g_in[:, 1, :])
                    g = pipe.intermediate_tile([P, 2, H], adt, name=f"g{li}")
                    if li >= n_layers:  # ablation: copy-through this mid layer
                        nc.scalar.copy(g, g_in)
                        return prev[:0] + (g,) + prev[1:]
                    pTf = ppt.tile([P, PTW], adt, tag="pT")
                    rows = (None, None)
                    if fold and negmT_in is not None:
                        rows = (negmT_in[0:1, :], negmT_in[32:33, :])
                    pz_o = matmuls(srcs[0], li, rows[0], pTf)
                    pz_g = matmuls(srcs[1], li, rows[1], pTf)
                    sums = mp.tile([P, 2], fp32, tag="sums")
                    gelu_pair(pz_o, pz_g, g, rstd_in, sums)
                    if ln_lite:
                        return (g,)
                    rstd, nmt = ln_stats(g, sums, pz_o, pz_g, pTf)
                    if fold:
                        negmT = pipe.intermediate_tile([33, P], adt,
                                                       name=f"nmT{li}")
                        nc.scalar.copy(negmT[0:1, :], nmt[0:1, H:H + P])
                        nc.scalar.copy(negmT[32:33, :], nmt[32:33, H:H + P])
                        return (g, pipe_keep(pipe, rstd, f"rstd{li}"), negmT)
                    return (g,)
                return st

            def st_l3(pipe, iv, prev):
                g_in = prev[0]
                rstd_in = prev[1] if len(prev) > 1 else None
                negmT_in = prev[2] if len(prev) > 2 else None
                phis = pipe.intermediate_tile([P, H], sdt, name="phis")
                ypr = pipe.intermediate_tile([P, H], sdt, name="ypr")
                pTf = ppt.tile([P, PTW], adt, tag="pT")
                rows = (None, None)
                if fold and negmT_in is not None:
                    rows = (negmT_in[0:1, :], negmT_in[32:33, :])
                pz = matmuls(g_in[:, 0, :], 3, rows[0], pTf)
                if fold and rstd_in is not None:
                    nc.scalar.mul(phis, pz, rstd_in[:, 0:1])
                else:
                    nc.scalar.copy(phis, pz)
                pzg = matmuls(g_in[:, 1, :], 3, rows[1], pTf)
                if fold and rstd_in is not None:
                    nc.vector.scalar_tensor_tensor(
                        out=ypr, in0=pzg, scalar=rstd_in[:, 1:2], in1=phis,
                        op0=OP.mult, op1=OP.max)
                else:
                    nc.vector.tensor_tensor(out=ypr, in0=phis, in1=pzg,
                                            op=OP.max)
                return (phis, ypr)

            def st_sort_a(pipe, iv, prev):
                phis, ypr = prev
                bufA = pipe.intermediate_tile([P, 2 * H], sdt, name="bufA")
                bufB = pipe.intermediate_tile([P, 2 * H], sdt, name="bufB")
                emit_sort_pass(0, phis, ypr, bufA)
                cur, nxt = bufA, bufB
                for pidx in range(1, split_pass):
                    if pidx < n_passes:
                        emit_sort_pass(pidx, cur, None, nxt)
                    cur, nxt = nxt, cur
                return (bufA, bufB)

            def st_sort_b(pipe, iv, prev):
                bufA, bufB = prev
                cur, nxt = (bufB, bufA) if split_pass % 2 == 0 else (bufA, bufB)
                for pidx in range(split_pass, 15):
                    if pidx < n_passes:
                        emit_sort_pass(pidx, cur, None, nxt)
                    cur, nxt = nxt, cur
                fin = cur
                fv = fin.rearrange("p (h g e) -> p h g e", h=2, e=DPC)
                # coupling: u[i] <- max(u[i], v[i-1]) for i>=1, in place
                nc.vector.tensor_tensor(out=fv[:, 0, :, 1:DPC],
                                        in0=fv[:, 0, :, 1:DPC],
                                        in1=fv[:, 1, :, 0:DPC - 1], op=OP.max)
                red = sp.tile([P, 2, NCOMP], fp32, tag="red")
                nc.vector.tensor_reduce(out=red, in_=fv,
                                        axis=mybir.AxisListType.X, op=OP.add)
                res = sp.tile([P, 1], fp32, tag="res")
                if fuse_tail:
                    comp = sp.tile([P, NCOMP], fp32, tag="comp")
                    cs = sp.tile([P, 1], fp32, tag="cs")
                    cm = sp.tile([P, 1], fp32, tag="cm")
                    nc.vector.tensor_tensor_reduce(
                        out=comp, in0=red[:, 1, :], in1=red[:, 0, :],
                        scale=1.0, scalar=0.0, op0=OP.subtract, op1=OP.add,
                        accum_out=cs)
                    comp2 = sp.tile([P, NCOMP], fp32, tag="comp2")
                    nc.vector.tensor_tensor_reduce(
                        out=comp2, in0=red[:, 1, :], in1=red[:, 0, :],
                        scale=1.0, scalar=-3e38, op0=OP.subtract, op1=OP.max,
                        accum_out=cm)
                else:
                    comp = sp.tile([P, NCOMP], fp32, tag="comp")
                    nc.vector.tensor_tensor(out=comp, in0=red[:, 1, :],
                                            in1=red[:, 0, :], op=OP.subtract)
                    cs = sp.tile([P, 1], fp32, tag="cs")
                    nc.vector.tensor_reduce(out=cs, in_=comp,
                                            axis=mybir.AxisListType.X,
                                            op=OP.add)
                    cm = sp.tile([P, 1], fp32, tag="cm")
                    nc.vector.tensor_reduce(out=cm, in_=comp,
                                            axis=mybir.AxisListType.X,
                                            op=OP.max)
                nc.vector.tensor_scalar(out=res, in0=cs, scalar1=avec[:, 0:1],
                                        scalar2=None, op0=OP.mult)
                nc.vector.scalar_tensor_tensor(out=res, in0=cm,
                                               scalar=avec[:, 1:2], in1=res,
                                               op0=OP.mult, op1=OP.add)
                nc.sync.dma_start(out=out_v[iv], in_=res[:, 0:1])

            stages = [st_load, mk_layer(0), mk_layer(1), mk_layer(2), st_l3,
                      st_sort_a, st_sort_b]

            def run_pipe():
                he = (mybir.EngineType.PE, mybir.EngineType.DVE,
                      mybir.EngineType.Activation, mybir.EngineType.SP,
                      mybir.EngineType.Pool) if hints else ()
                tc.For_i_pipelined(stages, 0, nt, 1, pool=pipe_pool,
                                   unroll=unroll, staged_num_bufs=stage_bufs,
                                   hint_engines=he)

            if repeats == 1:
                run_pipe()
            else:
                with tc.For_i(0, repeats, 1):
                    run_pipe()

    nc.finalize()
    return nc


# ---------------------------------------------------------------- host wrapper
def _prep_host(inputs, mm16=True):
    """Fold LN affine params into the following layer's weights; build avec."""
    f32 = np.float32
    adt = np.float16 if mm16 else f32
    W0 = np.asarray(inputs["W0"], f32)
    b0 = np.asarray(inputs["b0"], f32)
    w, b = [W0], [b0]
    for i in (0, 1, 2):
        s = np.asarray(inputs[f"ln{i}_s"], f32)
        t = np.asarray(inputs[f"ln{i}_b"], f32)
        Wn = np.asarray(inputs[("W1", "W2", "W3")[i]], f32)
        bn = np.asarray(inputs[("b1", "b2", "b3")[i]], f32)
        w.append(s[:, None] * Wn)
        b.append(bn + t @ Wn)
    bs = np.stack(b, 0)  # [4, 512]
    has_bias = bool(np.any(np.abs(bs) > 0))
    csw = np.stack([w[1].sum(0), w[2].sum(0), w[3].sum(0)], 0)  # [3, 512]
    alpha = float(np.asarray(inputs["alpha"]))
    a = 1.0 / (1.0 + np.exp(-alpha))
    avec = np.empty((P, 2), f32)
    avec[:, 0] = a / NCOMP
    avec[:, 1] = 1.0 - a
    return ([x.astype(adt) for x in w], bs.astype(adt), csw.astype(adt),
            avec, has_bias)


def _probe_devices():
    """Poke every core with a tiny op; retries to shake off a stale
    NRT_EXEC_UNIT_UNRECOVERABLE state left by a previous process."""
    import jax
    import jax.numpy as jnp

    for attempt in range(3):
        try:
            for d in jax.devices()[:NCORES]:
                jnp.zeros((1,), jnp.float32, device=d).block_until_ready()
            return
        except Exception:
            if attempt == 2:
                raise


def run_on_device(inputs, rows_total=B, trace=False, repeats=1, **build_kw):
    """Shard, run on 8 cores, gather. Returns (out [rows_total], results obj)."""
    from concourse.bass_utils import run_bass_kernel_spmd

    _probe_devices()

    mm16 = bool(build_kw.get("mm16", 1))
    (w0, w1, w2, w3), bs, csw, avec, has_bias = _prep_host(inputs, mm16=mm16)
    build_kw.setdefault("has_bias", int(has_bias))

    rows_core = rows_total // NCORES
    key = (rows_core, repeats, tuple(sorted(build_kw.items())))
    if key not in _CACHE:
        _CACHE[key] = build_nc(rows_core, repeats=repeats, **build_kw)
    nc = _CACHE[key]

    adt = np.float16 if mm16 else np.float32
    ob = np.ascontiguousarray(
        np.asarray(inputs["observations"])[:rows_total].astype(adt))
    gl = np.ascontiguousarray(
        np.asarray(inputs["goals"])[:rows_total].astype(adt))
    in_maps = []
    for c in range(NCORES):
        sl = slice(c * rows_core, (c + 1) * rows_core)
        m = {
            "observations": ob[sl], "goals": gl[sl],
            "w0": w0, "w1": w1, "w2": w2, "w3": w3, "csw": csw, "avec": avec,
        }
        if has_bias:
            m["bs"] = bs
        in_maps.append(m)
    r = run_bass_kernel_spmd(nc, in_maps, list(range(NCORES)), trace=trace)
    outp = np.concatenate([r.results[c]["out"] for c in range(NCORES)])
    return outp, r


def kernel(**inputs):
    out, _ = run_on_device(inputs)
    return out.astype(np.float32)
